# revision 18
# baseline (speedup 1.0000x reference)
"""Trainium2 Bass kernel for a chain of 20 radial flows on [8388608, 4] fp32.

Strategy: pure data parallel over 8 NeuronCores. Each core gets S = B/8
samples in a *fully packed per-component* layout: four fp16 tiles
d_c [128, S/128] (c = 0..3) holding d = x - x0_k (the flow offset),
sample s lives at (p = s // (S/128), f = s % (S/128)). fp16 (e5m10)
keeps the 16-bit 2x DVE modes while staying within the 2e-2 error
budget (bf16 does not: the 20-flow chain amplifies rounding ~13x).

Per flow k (sequential chain), on CH-sample chunks:
  sq_c = d_c^2                   ACT Square / DVE TT (2x) / GPSIMD TT,
                                 split per (comp, chunk) by ENG tables
  r2   = sum_c sq_c              PE: accumulating identity matmuls
  x    = sqrt(r2/beta^2)         ACT (PSUM src), x = r/|beta|
  w    = x*sgn + alpha/beta      DVE tensor_scalar (2x, AP scalars)
  m    = 1 + 1/w                 custom DVE op (bit-hack reciprocal), fp16
  t_c  = d_c*m                   DVE TT (fp16 2x) / GPSIMD TT
  d_c  = t_c + (x0_k - x0_{k+1}) DVE tensor_scalar add (fp16); the last
                                 flow adds x0_19 instead = output
Input prep: d_0 = x - x0_0 via one tensor_scalar pass after the fp16 DMA.
Output fp16, host upcasts to fp32.
"""

import sys

if "/opt/trn_rl_repo" not in sys.path:
    sys.path.insert(0, "/opt/trn_rl_repo")

from contextlib import ExitStack

import numpy as np

import concourse.bass as bass
import concourse.tile as tile
from concourse import bacc, mybir

F32 = mybir.dt.float32
F16 = mybir.dt.float16

BATCH = 8388608
DIM = 4
N_FLOWS = 20
N_CORES = 8
S = BATCH // N_CORES          # samples per core
PK = S // 128                 # packed free-dim per component tile (8192)
CH = 2048                     # chunk free-dim
NCH = PK // CH                # chunks (4)

# engine split knobs: for each (comp, chunk), which engine runs the op.
# 'A' = ACT, 'V' = DVE, 'G' = GPSIMD
SQ_ENG = [["A"] * NCH, ["A"] * NCH, ["A", "A", "V", "V"], ["V"] * NCH]
SPAN = 4096                   # free-dim for the update ops
NSP = PK // SPAN              # spans (2)

# per-flow param columns in pr: dnext[4], ln(alpha), beta/alpha
PRC = 8  # stride
_DN, _LA, _BA = 0, 4, 5
_D0 = N_FLOWS * PRC  # -x0_0 columns (input prep)

_CACHE = {}

# Chebyshev-minimax quadratic seed for 1/x on the u = x*bitcast(~x) interval
# [-4.5, -4]: 1/x ~= bitcast(~x) * (c2*u^2 + c1*u + c0), max rel err 5.2e-5.
_RM_C = (-0.706758876, -0.166355887, -0.013040724)


def _radial_m_op():
    """out = bitcast(~x)*((C2*u + C1)*u + C0) + src1, u = x*bitcast(~x).

    With src1 = ones column: out = 1 + 1/Src0 to ~5e-5 — fuses the
    reciprocal and the +1 of m = 1 + beta/(alpha+r) into one 8-stage op."""
    from concourse import dve_ops
    from concourse.dve_spec import AluOp, Bin, Spec, Src0, Src1, lower
    from concourse.dve_spec import C0, C1, C2
    from concourse.dve_uop import DveOpSpec

    for op in dve_ops.OPS:
        if op.name == "RADIAL_M":
            return op
    _not = Bin(AluOp.BITWISE_NOT, Src0, Src0)
    _u = Src0 * _not
    body = _not * ((_u * C2 + C1) * _u + C0) + Src1

    def ref(in0, in1, c0, c1, c2):
        not_x = (~in0.view(np.int32)).view(np.float32)
        u = (in0 * not_x).astype(np.float32)
        return (not_x * ((u * c2 + c1) * u + c0) + in1).astype(np.float32)

    spec = Spec(body=body, reference=ref)
    row = max(dve_ops._SUB_OPCODE_FOR_NAME.values()) + 1
    assert row < 0x20
    dve_ops._SUB_OPCODE_FOR_NAME["RADIAL_M"] = row
    shas = {}
    for ver in ("v3", "v4"):
        uops = lower(spec, ver=ver)
        shas[ver] = DveOpSpec(
            name="RADIAL_M", opcode=row, uops=uops, rd1_en=True).sha(ver)
    op = dve_ops.DveOp("RADIAL_M", spec, subdim=False, uops_sha=shas)
    dve_ops.OPS.append(op)
    dve_ops.CUSTOM_DVE_SPECS["RADIAL_M"] = spec
    return op


def _patch_ldw_opt():
    from concourse import bass_utils as _bu
    if getattr(_bu, "_ldw_patched", False):
        return
    _orig = _bu.bir_verify_and_optimise

    def _patched(*a, **kw):
        import unittest.mock as _m
        real_run = _bu.run_command

        def run2(cmd, **k):
            cmd = [c.replace("--enable-ldw-opt=false", "--enable-ldw-opt=true")
                   if isinstance(c, str) else c for c in cmd]
            return real_run(cmd, **k)
        with _m.patch.object(_bu, "run_command", run2):
            return _orig(*a, **kw)
    _bu._ldw_patched = True  # ldw-opt pass crashes walrus; keep disabled


def _build_program(n_flows=N_FLOWS):
    _patch_ldw_opt()
    nc = bacc.Bacc("TRN2", target_bir_lowering=False, debug=False,
                   num_devices=N_CORES)
    xt = nc.dram_tensor("xt", [DIM, 128, PK], F16, kind="ExternalInput").ap()
    wi = nc.dram_tensor("wi", [128, 128], F16, kind="ExternalInput").ap()
    pr = nc.dram_tensor("pr", [128, _D0 + DIM], F32,
                        kind="ExternalInput").ap()
    ot = nc.dram_tensor("ot", [DIM, 128, PK], F16, kind="ExternalOutput").ap()

    AL = mybir.AluOpType
    ACT = mybir.ActivationFunctionType

    def col(i):  # per-partition scalar AP from the params tile
        return pr_t[:, i:i + 1]

    with tile.TileContext(nc) as tc, ExitStack() as ctx:
        singles = ctx.enter_context(tc.tile_pool(name="singles", bufs=1))
        state = ctx.enter_context(tc.tile_pool(name="state", bufs=1))
        sq_pool = ctx.enter_context(tc.tile_pool(name="sq", bufs=2))
        lg_pool = ctx.enter_context(tc.tile_pool(name="lg", bufs=1))
        s_pool = ctx.enter_context(tc.tile_pool(name="s", bufs=2))
        o_pool = ctx.enter_context(tc.tile_pool(name="o", bufs=1))
        r2_pool = ctx.enter_context(
            tc.tile_pool(name="r2", bufs=2, space="PSUM"))

        pr_t = singles.tile([128, _D0 + DIM], F32)
        nc.sync.dma_start(pr_t[:], pr[:])
        wi_t = singles.tile([128, 128], F16)
        nc.sync.dma_start(wi_t[:], wi[:])

        d = [state.tile([128, PK], F16, tag=f"d{c}", name=f"d{c}")
             for c in range(DIM)]
        for c in range(DIM):
            nc.sync.dma_start(d[c][:], xt[c])
        for c in range(DIM):  # d_0 = x - x0_0
            nc.vector.tensor_scalar(
                out=d[c][:], in0=d[c][:], scalar1=col(_D0 + c), scalar2=None,
                op0=AL.add)

        for k in range(n_flows):
            pk = k * PRC
            last = (k == n_flows - 1)
            sg = {}
            for t in range(NCH):
                sl = bass.ts(t, CH)
                r2 = r2_pool.tile([128, CH], F32)
                sq = {}
                for c in range(DIM):
                    sq[c] = sq_pool.tile([128, CH], F16, tag=f"sq{c}",
                                         name=f"sq{c}")
                    if SQ_ENG[c][t] == "A":
                        nc.scalar.activation(sq[c][:], d[c][:, sl],
                                             ACT.Square)
                    else:
                        nc.vector.tensor_tensor(
                            out=sq[c][:], in0=d[c][:, sl], in1=d[c][:, sl],
                            op=AL.mult)
                sp01 = sq_pool.tile([128, CH], F16, tag="sp01",
                                    name="sp01")
                sp23 = sq_pool.tile([128, CH], F16, tag="sp23",
                                    name="sp23")
                nc.gpsimd.tensor_tensor(out=sp01[:], in0=sq[0][:],
                                        in1=sq[1][:], op=AL.add)
                nc.gpsimd.tensor_tensor(out=sp23[:], in0=sq[2][:],
                                        in1=sq[3][:], op=AL.add)
                for j in range(CH // 512):
                    js = bass.ts(j, 512)
                    for pp, t2 in ((sp01, False), (sp23, True)):
                        nc.tensor.matmul(
                            out=r2[:, js], lhsT=wi_t[:], rhs=pp[:, js],
                            start=(not t2), stop=t2)
                # L = ln(r2), batched per flow before the sigmoids
                lg = lg_pool.tile([128, CH], F32, tag=f"lg{t % 2}",
                                  name=f"lg{t % 2}")
                nc.scalar.activation(lg[:], r2[:], ACT.Ln)
                sg[t] = lg
            s = {}
            for t in range(NCH):
                sp, half = t // (SPAN // CH), t % (SPAN // CH)
                if half == 0:
                    s[sp] = s_pool.tile([128, SPAN], F16, tag=f"s{sp}",
                                        name=f"s{sp}")
                # s = sigmoid(-L/2 + ln a) = a/(a+r)
                nc.scalar.activation(
                    s[sp][:, bass.ts(half, CH)], sg[t][:], ACT.Sigmoid,
                    bias=col(pk + _LA), scale=-0.5)
            for sp in range(NSP):
                ssl = bass.ts(sp, SPAN)
                # m = 1 + (beta/alpha) * s, in place over s
                nc.vector.tensor_scalar(
                    out=s[sp][:], in0=s[sp][:], scalar1=col(pk + _BA),
                    scalar2=1.0, op0=AL.mult, op1=AL.add)
                for c in range(DIM):
                    nc.vector.tensor_tensor(
                        out=d[c][:, ssl], in0=d[c][:, ssl], in1=s[sp][:],
                        op=AL.mult)
                for c in range(DIM):
                    dst = d[c][:, ssl]
                    if last:
                        dst = o_pool.tile([128, SPAN], F16, tag=f"o{c}",
                                          name=f"o{c}")[:]
                    nc.vector.tensor_scalar(
                        out=dst, in0=d[c][:, ssl], scalar1=col(pk + _DN + c),
                        scalar2=None, op0=AL.add)
                    if last:
                        nc.sync.dma_start(ot[c][:, ssl], dst)

    nc.compile()
    return nc


def _host_params(x0s, alpha_primes, beta_primes, n_flows=N_FLOWS):
    x0s = np.asarray(x0s, np.float32)
    sp_a = np.logaddexp(np.float32(0.0), np.asarray(alpha_primes, np.float32))
    sp_b = np.logaddexp(np.float32(0.0), np.asarray(beta_primes, np.float32))
    alpha = sp_a.astype(np.float32)
    beta = (-alpha + sp_b).astype(np.float32)

    pr = np.zeros((128, _D0 + DIM), np.float32)
    for k in range(n_flows):
        # post-update shift: x0_k - x0_{k+1}; after the last flow, + x0_19
        dnext = (x0s[k] - x0s[k + 1]) if k < n_flows - 1 else x0s[n_flows - 1]
        bk = beta[k] if beta[k] != 0.0 else np.float32(1e-30)
        pk = k * PRC
        pr[:, pk + _DN:pk + _DN + DIM] = dnext[None, :]
        pr[:, pk + _LA] = np.log(alpha[k])
        pr[:, pk + _BA] = bk / alpha[k]
    pr[:, _D0:_D0 + DIM] = -x0s[0][None, :]

    wi = np.eye(128, dtype=np.float16)
    return pr, wi


def kernel(X, x0s, alpha_primes, beta_primes):
    from concourse.bass_utils import run_bass_kernel_spmd

    X = np.asarray(X, np.float32)
    pr, wi = _host_params(x0s, alpha_primes, beta_primes)

    if "nc" not in _CACHE:
        _CACHE["nc"] = _build_program()
    nc = _CACHE["nc"]

    in_maps = []
    for c in range(N_CORES):
        shard = X[c * S:(c + 1) * S]
        xt = np.ascontiguousarray(shard.T).astype(np.float16)
        in_maps.append({
            "xt": xt.reshape(DIM, 128, PK),
            "wi": wi,
            "pr": pr,
        })
    # The device occasionally throws a spurious NRT_EXEC_UNIT_UNRECOVERABLE
    # and recovers by the next run; retry rather than failing the call.
    res = None
    for attempt in range(3):
        try:
            res = run_bass_kernel_spmd(nc, in_maps, list(range(N_CORES)))
            break
        except Exception:
            if attempt == 2:
                raise
    out = np.empty((BATCH, DIM), np.float32)
    for c in range(N_CORES):
        o = res.results[c]["ot"].astype(np.float32).reshape(DIM, S)
        out[c * S:(c + 1) * S] = o.T
    return out


# revision 20
# speedup vs baseline: 1.3555x; 1.3555x over previous
"""Trainium2 Bass kernel for a chain of 20 radial flows on [8388608, 4] fp32.

Strategy: pure data parallel over 8 NeuronCores. Each core gets S = B/8
samples in a *fully packed per-component* layout: four fp16 tiles
d_c [128, S/128] (c = 0..3) holding d = x - x0_k (the flow offset),
sample s lives at (p = s // (S/128), f = s % (S/128)). fp16 (e5m10)
keeps the 16-bit 2x DVE modes while staying within the 2e-2 error
budget (bf16 does not: the 20-flow chain amplifies rounding ~13x).

Per flow k (sequential chain), on CH-sample chunks:
  sq_c = d_c^2                   ACT Square / DVE TT (2x) / GPSIMD TT,
                                 split per (comp, chunk) by ENG tables
  r2   = sum_c sq_c              PE: accumulating identity matmuls
  x    = sqrt(r2/beta^2)         ACT (PSUM src), x = r/|beta|
  w    = x*sgn + alpha/beta      DVE tensor_scalar (2x, AP scalars)
  m    = 1 + 1/w                 custom DVE op (bit-hack reciprocal), fp16
  t_c  = d_c*m                   DVE TT (fp16 2x) / GPSIMD TT
  d_c  = t_c + (x0_k - x0_{k+1}) DVE tensor_scalar add (fp16); the last
                                 flow adds x0_19 instead = output
Input prep: d_0 = x - x0_0 via one tensor_scalar pass after the fp16 DMA.
Output fp16, host upcasts to fp32.
"""

import sys

if "/opt/trn_rl_repo" not in sys.path:
    sys.path.insert(0, "/opt/trn_rl_repo")

from contextlib import ExitStack

import numpy as np

import concourse.bass as bass
import concourse.tile as tile
from concourse import bacc, mybir

F32 = mybir.dt.float32
F16 = mybir.dt.float16

BATCH = 8388608
DIM = 4
N_FLOWS = 20
N_CORES = 8
S = BATCH // N_CORES          # samples per core
PK = S // 128                 # packed free-dim per component tile (8192)
CH = 2048                     # chunk free-dim
NCH = PK // CH                # chunks (4)

# engine split knobs: for each (comp, chunk), which engine runs the op.
# 'A' = ACT, 'V' = DVE, 'G' = GPSIMD
SQ_ENG = [["A"] * NCH, ["A"] * NCH, ["A", "A", "V", "V"], ["V"] * NCH]
SPAN = 4096                   # free-dim for the update ops
NSP = PK // SPAN              # spans (2)

# per-flow param columns in pr: dnext[4], ln(alpha), beta/alpha
PRC = 8  # stride
_DN, _LA, _BA = 0, 4, 5
_D0 = N_FLOWS * PRC  # -x0_0 columns (input prep)

_CACHE = {}

# Chebyshev-minimax quadratic seed for 1/x on the u = x*bitcast(~x) interval
# [-4.5, -4]: 1/x ~= bitcast(~x) * (c2*u^2 + c1*u + c0), max rel err 5.2e-5.
_RM_C = (-0.706758876, -0.166355887, -0.013040724)


def _radial_m_op():
    """out = bitcast(~x)*((C2*u + C1)*u + C0) + src1, u = x*bitcast(~x).

    With src1 = ones column: out = 1 + 1/Src0 to ~5e-5 — fuses the
    reciprocal and the +1 of m = 1 + beta/(alpha+r) into one 8-stage op."""
    from concourse import dve_ops
    from concourse.dve_spec import AluOp, Bin, Spec, Src0, Src1, lower
    from concourse.dve_spec import C0, C1, C2
    from concourse.dve_uop import DveOpSpec

    for op in dve_ops.OPS:
        if op.name == "RADIAL_M":
            return op
    _not = Bin(AluOp.BITWISE_NOT, Src0, Src0)
    _u = Src0 * _not
    body = _not * ((_u * C2 + C1) * _u + C0) + Src1

    def ref(in0, in1, c0, c1, c2):
        not_x = (~in0.view(np.int32)).view(np.float32)
        u = (in0 * not_x).astype(np.float32)
        return (not_x * ((u * c2 + c1) * u + c0) + in1).astype(np.float32)

    spec = Spec(body=body, reference=ref)
    row = max(dve_ops._SUB_OPCODE_FOR_NAME.values()) + 1
    assert row < 0x20
    dve_ops._SUB_OPCODE_FOR_NAME["RADIAL_M"] = row
    shas = {}
    for ver in ("v3", "v4"):
        uops = lower(spec, ver=ver)
        shas[ver] = DveOpSpec(
            name="RADIAL_M", opcode=row, uops=uops, rd1_en=True).sha(ver)
    op = dve_ops.DveOp("RADIAL_M", spec, subdim=False, uops_sha=shas)
    dve_ops.OPS.append(op)
    dve_ops.CUSTOM_DVE_SPECS["RADIAL_M"] = spec
    return op


def _patch_ldw_opt():
    from concourse import bass_utils as _bu
    if getattr(_bu, "_ldw_patched", False):
        return
    _orig = _bu.bir_verify_and_optimise

    def _patched(*a, **kw):
        import unittest.mock as _m
        real_run = _bu.run_command

        def run2(cmd, **k):
            cmd = [c.replace("--enable-ldw-opt=false", "--enable-ldw-opt=true")
                   if isinstance(c, str) else c for c in cmd]
            return real_run(cmd, **k)
        with _m.patch.object(_bu, "run_command", run2):
            return _orig(*a, **kw)
    _bu._ldw_patched = True  # ldw-opt pass crashes walrus; keep disabled


def _build_program(n_flows=N_FLOWS):
    _patch_ldw_opt()
    nc = bacc.Bacc("TRN2", target_bir_lowering=False, debug=False,
                   num_devices=N_CORES)
    xt = nc.dram_tensor("xt", [DIM, 128, PK], F16, kind="ExternalInput").ap()
    wi = nc.dram_tensor("wi", [128, 128], F16, kind="ExternalInput").ap()
    pr = nc.dram_tensor("pr", [128, _D0 + DIM], F32,
                        kind="ExternalInput").ap()
    ot = nc.dram_tensor("ot", [DIM, 128, PK], F16, kind="ExternalOutput").ap()

    AL = mybir.AluOpType
    ACT = mybir.ActivationFunctionType

    def col(i):  # per-partition scalar AP from the params tile
        return pr_t[:, i:i + 1]

    with tile.TileContext(nc) as tc, ExitStack() as ctx:
        singles = ctx.enter_context(tc.tile_pool(name="singles", bufs=1))
        state = ctx.enter_context(tc.tile_pool(name="state", bufs=1))
        sq_pool = ctx.enter_context(tc.tile_pool(name="sq", bufs=2))
        lg_pool = ctx.enter_context(tc.tile_pool(name="lg", bufs=2))
        s_pool = ctx.enter_context(tc.tile_pool(name="s", bufs=2))
        o_pool = ctx.enter_context(tc.tile_pool(name="o", bufs=1))
        r2_pool = ctx.enter_context(
            tc.tile_pool(name="r2", bufs=2, space="PSUM"))

        pr_t = singles.tile([128, _D0 + DIM], F32)
        nc.sync.dma_start(pr_t[:], pr[:])
        wi_t = singles.tile([128, 128], F16)
        nc.sync.dma_start(wi_t[:], wi[:])

        d = [state.tile([128, PK], F16, tag=f"d{c}", name=f"d{c}")
             for c in range(DIM)]
        for c in range(DIM):
            nc.sync.dma_start(d[c][:], xt[c])
        for c in range(DIM):  # d_0 = x - x0_0
            nc.vector.tensor_scalar(
                out=d[c][:], in0=d[c][:], scalar1=col(_D0 + c), scalar2=None,
                op0=AL.add)

        for k in range(n_flows):
            pk = k * PRC
            last = (k == n_flows - 1)
            sg = {}
            for t in range(NCH):
                sl = bass.ts(t, CH)
                r2 = r2_pool.tile([128, CH], F32)
                sq = {}
                for c in range(DIM):
                    sq[c] = sq_pool.tile([128, CH], F16, tag=f"sq{c}",
                                         name=f"sq{c}")
                    if SQ_ENG[c][t] == "A":
                        nc.scalar.activation(sq[c][:], d[c][:, sl],
                                             ACT.Square)
                    else:
                        nc.vector.tensor_tensor(
                            out=sq[c][:], in0=d[c][:, sl], in1=d[c][:, sl],
                            op=AL.mult)
                for j in range(CH // 512):
                    js = bass.ts(j, 512)
                    for c in range(DIM):
                        nc.tensor.matmul(
                            out=r2[:, js], lhsT=wi_t[:], rhs=sq[c][:, js],
                            start=(c == 0), stop=(c == DIM - 1))
                # L = ln(r2), batched per flow before the sigmoids
                lg = lg_pool.tile([128, CH], F32, tag=f"lg{t % 2}",
                                  name=f"lg{t % 2}")
                nc.scalar.activation(lg[:], r2[:], ACT.Ln)
                sg[t] = lg
            s = {}
            for t in range(NCH):
                sp, half = t // (SPAN // CH), t % (SPAN // CH)
                if half == 0:
                    s[sp] = s_pool.tile([128, SPAN], F16, tag=f"s{sp}",
                                        name=f"s{sp}")
                # s = sigmoid(-L/2 + ln a) = a/(a+r)
                nc.scalar.activation(
                    s[sp][:, bass.ts(half, CH)], sg[t][:], ACT.Sigmoid,
                    bias=col(pk + _LA), scale=-0.5)
            for sp in range(NSP):
                ssl = bass.ts(sp, SPAN)
                # m = 1 + (beta/alpha) * s, in place over s
                nc.vector.tensor_scalar(
                    out=s[sp][:], in0=s[sp][:], scalar1=col(pk + _BA),
                    scalar2=1.0, op0=AL.mult, op1=AL.add)
                for c in range(DIM):
                    nc.vector.tensor_tensor(
                        out=d[c][:, ssl], in0=d[c][:, ssl], in1=s[sp][:],
                        op=AL.mult)
                for c in range(DIM):
                    dst = d[c][:, ssl]
                    if last:
                        dst = o_pool.tile([128, SPAN], F16, tag=f"o{c}",
                                          name=f"o{c}")[:]
                    nc.vector.tensor_scalar(
                        out=dst, in0=d[c][:, ssl], scalar1=col(pk + _DN + c),
                        scalar2=None, op0=AL.add)
                    if last:
                        nc.sync.dma_start(ot[c][:, ssl], dst)

    nc.compile()
    return nc


def _host_params(x0s, alpha_primes, beta_primes, n_flows=N_FLOWS):
    x0s = np.asarray(x0s, np.float32)
    sp_a = np.logaddexp(np.float32(0.0), np.asarray(alpha_primes, np.float32))
    sp_b = np.logaddexp(np.float32(0.0), np.asarray(beta_primes, np.float32))
    alpha = sp_a.astype(np.float32)
    beta = (-alpha + sp_b).astype(np.float32)

    pr = np.zeros((128, _D0 + DIM), np.float32)
    for k in range(n_flows):
        # post-update shift: x0_k - x0_{k+1}; after the last flow, + x0_19
        dnext = (x0s[k] - x0s[k + 1]) if k < n_flows - 1 else x0s[n_flows - 1]
        bk = beta[k] if beta[k] != 0.0 else np.float32(1e-30)
        pk = k * PRC
        pr[:, pk + _DN:pk + _DN + DIM] = dnext[None, :]
        pr[:, pk + _LA] = np.log(alpha[k])
        pr[:, pk + _BA] = bk / alpha[k]
    pr[:, _D0:_D0 + DIM] = -x0s[0][None, :]

    wi = np.eye(128, dtype=np.float16)
    return pr, wi


def kernel(X, x0s, alpha_primes, beta_primes):
    from concourse.bass_utils import run_bass_kernel_spmd

    X = np.asarray(X, np.float32)
    pr, wi = _host_params(x0s, alpha_primes, beta_primes)

    if "nc" not in _CACHE:
        _CACHE["nc"] = _build_program()
    nc = _CACHE["nc"]

    in_maps = []
    for c in range(N_CORES):
        shard = X[c * S:(c + 1) * S]
        xt = np.ascontiguousarray(shard.T).astype(np.float16)
        in_maps.append({
            "xt": xt.reshape(DIM, 128, PK),
            "wi": wi,
            "pr": pr,
        })
    # The device occasionally throws a spurious NRT_EXEC_UNIT_UNRECOVERABLE
    # and recovers by the next run; retry rather than failing the call.
    res = None
    for attempt in range(3):
        try:
            res = run_bass_kernel_spmd(nc, in_maps, list(range(N_CORES)))
            break
        except Exception:
            if attempt == 2:
                raise
    out = np.empty((BATCH, DIM), np.float32)
    for c in range(N_CORES):
        o = res.results[c]["ot"].astype(np.float32).reshape(DIM, S)
        out[c * S:(c + 1) * S] = o.T
    return out


# revision 22
# speedup vs baseline: 1.4440x; 1.0653x over previous
"""Trainium2 Bass kernel for a chain of 20 radial flows on [8388608, 4] fp32.

Fully packed per-component fp16 layout; sigmoid identity for m:
  m = 1 + beta/(alpha+r) = 1 + (beta/alpha) * sigmoid(ln(alpha) - ln(r^2)/2)
Per flow: sq (ACT/DVE split) -> PE identity-accumulate r2 (2048-chunks)
-> ACT Ln (PSUM src) -> ACT Sigmoid -> DVE tensor_scalar m = s*(b/a)+1
(fp16 4x, in place) -> DVE TT t = d*m (fp16 2x, in place) -> DVE TS
d' = t + dnext (fp16 4x, in place; the last flow adds x0_19 = output).
Deep tile-pool buffering (sq/s bufs=3) decouples the flows; the last
flow DMAs straight out of the d state tiles.
"""

import sys

if "/opt/trn_rl_repo" not in sys.path:
    sys.path.insert(0, "/opt/trn_rl_repo")

from contextlib import ExitStack

import numpy as np

import concourse.bass as bass
import concourse.tile as tile
from concourse import bacc, mybir

F32 = mybir.dt.float32
F16 = mybir.dt.float16
BF16 = mybir.dt.bfloat16

BATCH = 8388608
DIM = 4
N_FLOWS = 20
N_CORES = 8
S = BATCH // N_CORES
PK = S // 128                 # 8192
CH = 2048
NCH = PK // CH                # 4
SPAN = 4096                   # update op width
NSP = PK // SPAN              # 2

SQ_ENG = [["A"] * NCH, ["A", "A", "V", "V"], ["A", "A", "V", "V"], ["V"] * NCH]

PRC = 8
_DN, _LA, _BA = 0, 4, 5
_D0 = N_FLOWS * PRC

_CACHE = {}


def _build_program(n_flows=N_FLOWS):
    nc = bacc.Bacc("TRN2", target_bir_lowering=False, debug=False,
                   num_devices=N_CORES)
    xt = nc.dram_tensor("xt", [DIM, 128, PK], F16, kind="ExternalInput").ap()
    wi = nc.dram_tensor("wi", [128, 128], BF16, kind="ExternalInput").ap()
    pr = nc.dram_tensor("pr", [128, _D0 + DIM], F32,
                        kind="ExternalInput").ap()
    ot = nc.dram_tensor("ot", [DIM, 128, PK], F16, kind="ExternalOutput").ap()

    AL = mybir.AluOpType
    ACT = mybir.ActivationFunctionType

    def col(i):
        return pr_t[:, i:i + 1]

    with tile.TileContext(nc) as tc, ExitStack() as ctx:
        singles = ctx.enter_context(tc.tile_pool(name="singles", bufs=1))
        state = ctx.enter_context(tc.tile_pool(name="state", bufs=1))
        sq_pool = ctx.enter_context(tc.tile_pool(name="sq", bufs=3))
        lg_pool = ctx.enter_context(tc.tile_pool(name="lg", bufs=2))
        s_pool = ctx.enter_context(tc.tile_pool(name="s", bufs=3))
        r2_pool = ctx.enter_context(
            tc.tile_pool(name="r2", bufs=2, space="PSUM"))

        pr_t = singles.tile([128, _D0 + DIM], F32)
        nc.sync.dma_start(pr_t[:], pr[:])
        wi_t = singles.tile([128, 128], BF16)
        nc.sync.dma_start(wi_t[:], wi[:])

        d = [state.tile([128, PK], F16, tag=f"d{c}", name=f"d{c}")
             for c in range(DIM)]
        for c in range(DIM):
            nc.sync.dma_start(d[c][:], xt[c])
        for c in range(DIM):  # d_0 = x - x0_0
            nc.vector.tensor_scalar(
                out=d[c][:], in0=d[c][:], scalar1=col(_D0 + c), scalar2=None,
                op0=AL.add)

        for k in range(n_flows):
            pk = k * PRC
            last = (k == n_flows - 1)
            sg = {}
            for t in range(NCH):
                sl = bass.ts(t, CH)
                r2 = r2_pool.tile([128, CH], F32)
                sq = {}
                for c in range(DIM):
                    sq[c] = sq_pool.tile([128, CH], BF16, tag=f"sq{c}",
                                         name=f"sq{c}")
                    if SQ_ENG[c][t] == "A":
                        nc.scalar.activation(sq[c][:], d[c][:, sl],
                                             ACT.Square)
                    else:
                        nc.vector.tensor_tensor(
                            out=sq[c][:], in0=d[c][:, sl], in1=d[c][:, sl],
                            op=AL.mult)
                for j in range(CH // 512):
                    js = bass.ts(j, 512)
                    for c in range(DIM):
                        nc.tensor.matmul(
                            out=r2[:, js], lhsT=wi_t[:], rhs=sq[c][:, js],
                            start=(c == 0), stop=(c == DIM - 1))
                lg = lg_pool.tile([128, CH], F32, tag=f"lg{t % 2}",
                                  name=f"lg{t % 2}")
                nc.scalar.activation(lg[:], r2[:], ACT.Ln)
                sg[t] = lg
            s = {}
            for t in range(NCH):
                sp, half = t // (SPAN // CH), t % (SPAN // CH)
                if half == 0:
                    s[sp] = s_pool.tile([128, SPAN], F16, tag=f"s{sp}",
                                        name=f"s{sp}")
                nc.scalar.activation(
                    s[sp][:, bass.ts(half, CH)], sg[t][:], ACT.Sigmoid,
                    bias=col(pk + _LA), scale=-0.5)
            for sp in range(NSP):
                ssl = bass.ts(sp, SPAN)
                # m = 1 + (beta/alpha)*s, in place (fp16 4x)
                nc.vector.tensor_scalar(
                    out=s[sp][:], in0=s[sp][:], scalar1=col(pk + _BA),
                    scalar2=1.0, op0=AL.mult, op1=AL.add)
                for c in range(DIM):
                    nc.vector.tensor_tensor(
                        out=d[c][:, ssl], in0=d[c][:, ssl], in1=s[sp][:],
                        op=AL.mult)
                for c in range(DIM):
                    nc.vector.tensor_scalar(
                        out=d[c][:, ssl], in0=d[c][:, ssl],
                        scalar1=col(pk + _DN + c), scalar2=None, op0=AL.add)
                    if last:
                        nc.sync.dma_start(ot[c][:, ssl], d[c][:, ssl])

    nc.compile()
    return nc


def _host_params(x0s, alpha_primes, beta_primes, n_flows=N_FLOWS):
    x0s = np.asarray(x0s, np.float32)
    sp_a = np.logaddexp(np.float32(0.0), np.asarray(alpha_primes, np.float32))
    sp_b = np.logaddexp(np.float32(0.0), np.asarray(beta_primes, np.float32))
    alpha = sp_a.astype(np.float32)
    beta = (-alpha + sp_b).astype(np.float32)

    pr = np.zeros((128, _D0 + DIM), np.float32)
    for k in range(n_flows):
        dnext = (x0s[k] - x0s[k + 1]) if k < n_flows - 1 else x0s[n_flows - 1]
        bk = beta[k] if beta[k] != 0.0 else np.float32(1e-30)
        pk = k * PRC
        pr[:, pk + _DN:pk + _DN + DIM] = dnext[None, :]
        pr[:, pk + _LA] = np.log(alpha[k])
        pr[:, pk + _BA] = bk / alpha[k]
    pr[:, _D0:_D0 + DIM] = -x0s[0][None, :]

    import ml_dtypes
    wi = np.eye(128, dtype=ml_dtypes.bfloat16)
    return pr, wi


def kernel(X, x0s, alpha_primes, beta_primes):
    from concourse.bass_utils import run_bass_kernel_spmd

    X = np.asarray(X, np.float32)
    pr, wi = _host_params(x0s, alpha_primes, beta_primes)

    if "nc" not in _CACHE:
        _CACHE["nc"] = _build_program()
    nc = _CACHE["nc"]

    in_maps = []
    for c in range(N_CORES):
        shard = X[c * S:(c + 1) * S]
        xtc = np.ascontiguousarray(shard.T).astype(np.float16)
        in_maps.append({"xt": xtc.reshape(DIM, 128, PK), "wi": wi, "pr": pr})
    res = None
    for attempt in range(3):
        try:
            res = run_bass_kernel_spmd(nc, in_maps, list(range(N_CORES)))
            break
        except Exception:
            if attempt == 2:
                raise
    out = np.empty((BATCH, DIM), np.float32)
    for c in range(N_CORES):
        o = res.results[c]["ot"].astype(np.float32).reshape(DIM, S)
        out[c * S:(c + 1) * S] = o.T
    return out
